# revision 41
# baseline (speedup 1.0000x reference)
"""5G Polar encoder (CRC11 + subchannel alloc + butterfly + interleave) on 8 trn2 cores.

The whole reference computation is GF(2)-linear in u:
    parity  = (u @ crc_gen) mod 2                       -> linear
    bits    = [u | parity] = u @ [I | crc_gen]          -> linear
    scatter x[:, info_pos] = bits                       -> column selection (linear)
    butterfly stages x ^= x[:, g[s]]                    -> linear over GF(2)
    out     = x[:, perm_out]                            -> column gather (linear)

So on the host we compose one binary matrix M [512, 1024] from the tiny index
tables (cheap uint8 ops), and the device kernel is a single fused
    y = (u @ M) mod 2
data-parallel over the batch: each of the 8 cores computes an [8192, 512] @
[512, 1024] matmul in fp8e4 with DoubleRow perf mode (exact: all values are
0/1, sums <= 523 accumulate in f32 PSUM).

Active design (VERSION=5, ~74us/core vs ~92.5us for the v1 baseline,
measured via NTFF; device clock state adds +-15% run-to-run):
  - PSUM singles [128,1024] f32 with bufs=4 (8 banks): 4-deep pipeline so
    eviction latency never stalls the PE (warm MM issue rate is 216ns;
    64 tiles x 4 MMs = 55.3us is the hard PE floor for fp8-DR).
  - Eviction = ONE op per tile, alternating engines (even tiles: ACT
    activation Copy, odd: DVE tensor_copy), PSUM f32 -> u8 with hardware
    saturation. Raw sums go out as u8 (8.4 MiB/core, half of i16); host
    computes &1. Saturation certificate: sums >255 would clip to exactly
    255, so if no 255 appears in the output the result is provably exact
    (real max here is ~210); otherwise kernel() reruns an i16 build.
  - No on-device AND/narrow at all: tensor_scalar bitVec ops can't cast
    (verifier) and the separate i16->i8 narrow costs 1225ns/pair on DVE,
    more than it saves.
  - 12 dummy warmup matmuls (no DMA deps) hold the PE's HAM clock gate at
    2.4GHz so real MMs start warm (cold MMs run at 1.2GHz, ~2x slower).
  - Input u streamed in chunks (128/256/512 then 2048 batch): first chunk
    lands ~1us after the DMA engines come up; 2048-wide chunks give 2KB
    DMA packet lines; chunk pool bufs=3 provides backpressure so early
    chunks aren't bandwidth-starved by late ones (concurrent DMAs share
    engines round-robin at packet granularity).

Earlier HW findings kept for reference: tensor_scalar `mod` and ACT `Sin`
don't work on HW; Pool (gpsimd) copies are ~4.25us/tile; pair-wide
[128,2048] evictions save fixed overhead but the 2-deep psum pipeline they
force costs more (periodic PE stalls); dma_start through a rearranged AP
regresses; fused AND+cast is rejected ("TSP bitVec op cannot do cast").
"""

import numpy as np
import ml_dtypes

N_CORES = 8
BS = 65536
K = 512          # u feature dim (contraction)
N = 1024         # output columns
SHARD = BS // N_CORES  # 8192 batch rows per core
P = 128
KT = K // P      # 4 k-tiles
NB = SHARD // P  # 64 batch tiles per core

FP8_NP = ml_dtypes.float8_e4m3

_nc_cache = {}


def build_M(crc_gen, info_pos, ind_gather, perm_out):
    """Compose the encoder into one GF(2) matrix M [K, N]: out = (u @ M) mod 2."""
    crc_gen = np.asarray(crc_gen)
    info_pos = np.asarray(info_pos)
    ind_gather = np.asarray(ind_gather)
    perm_out = np.asarray(perm_out)
    k, _ = crc_gen.shape
    nb, n1 = ind_gather.shape
    kp = info_pos.shape[0]
    C = (crc_gen.astype(np.int64) & 1).astype(np.uint8)
    B = np.concatenate([np.eye(k, dtype=np.uint8), C], axis=1)  # [k, kp]
    # scatter bits into columns; duplicate indices: last write wins (matches
    # jax/numpy .at[].set application order)
    col_src = np.full(n1, -1, np.int64)
    col_src[info_pos] = np.arange(kp)
    A = np.zeros((k, n1), np.uint8)
    valid = col_src >= 0
    A[:, valid] = B[:, col_src[valid]]
    for s in range(nb):
        A = A ^ A[:, ind_gather[s]]
    return A[:, perm_out]  # [k, n]


def _build_nc(reps=1, do_mm=True, do_evict=True, evict="pool",
              w1_act=64, w3_dve=0, ev_stage=3, u_chunks=1, wbufs=4,
              ks_outer=False):
    """evict modes:
    - "pool":    ACT f32->i16, DVE AND, Pool narrow i16->i8, DMA i8
    - "dve":     ACT f32->i16, DVE AND, DVE narrow i16->i8, DMA i8
    - "i16out":  ACT f32->i16, DVE AND, DMA out i16 (host takes low bits)
    - "dmacast": ACT f32->i16, DVE AND, gpsimd casting DMA i16->i8
    - "split":   W1 on ACT for w1_act tiles/64 else DVE; AND on DVE;
                 narrow on DVE for w3_dve tiles/64 else Pool; DMA i8
    """
    import concourse.tile as tile
    from concourse import bacc, mybir

    nc = bacc.Bacc("TRN2", target_bir_lowering=False, debug=False)
    fp8 = mybir.dt.float8e4
    f32 = mybir.dt.float32
    i16 = mybir.dt.int16
    i8 = mybir.dt.int8
    DR = mybir.MatmulPerfMode.DoubleRow

    # k-major 3D layouts: [p, ks, free] with global k = ks*128 + p (both
    # operands use the same mapping, so the contraction is correct).
    uT = nc.declare_dram_parameter("uT", [P, KT, SHARD], fp8, isOutput=False)
    mat = nc.declare_dram_parameter("mat", [P, KT, N], fp8, isOutput=False)
    y_dt = i16 if evict == "i16out" else i8
    y = nc.declare_dram_parameter("y", [SHARD, N], y_dt, isOutput=True)

    with tile.TileContext(nc) as tc:
        with (
            tc.tile_pool(name="consts", bufs=1) as cpool,
            tc.tile_pool(name="work", bufs=wbufs) as wpool,
            tc.tile_pool(name="outs", bufs=4) as opool,
            tc.tile_pool(name="psum", bufs=4, space="PSUM") as ppool,
        ):
            mt = cpool.tile([P, KT, N], fp8, tag="mt")
            nc.sync.dma_start(mt[:], mat[:])
            # chunk the big u load along batch so the first b-tile's matmuls
            # start after ~1/u_chunks of the 4MB has landed
            CW = SHARD // u_chunks
            uts = []
            for c in range(u_chunks):
                ut_c = cpool.tile([P, KT, CW], fp8, tag=f"ut{c}", name=f"ut{c}")
                nc.sync.dma_start(ut_c[:], uT[:, :, c * CW:(c + 1) * CW])
                uts.append(ut_c)
            ot_shared = None
            if evict == "outonly":
                ot_shared = cpool.tile([P, N], i8, tag="ot_shared")
                nc.any.memset(ot_shared[:], 0)
            ps_shared = None
            if not do_mm:
                ps_shared = ppool.tile([P, N], f32, tag="ps_shared")
                for h in range(2):
                    nc.tensor.matmul(
                        ps_shared[:, h * 512:(h + 1) * 512],
                        uts[0][:, 0:2, 0:P],
                        mt[:, 0:2, h * 512:(h + 1) * 512],
                        start=True, stop=True, perf_mode=DR,
                    )
            for i, b in enumerate(
                [b for _ in range(reps) for b in range(NB)]
            ):
                if do_mm:
                    ps = ppool.tile([P, N], f32, tag="ps", name="ps")
                else:
                    ps = ps_shared
                t16 = wpool.tile([P, N], i16, tag="t16")
                a16 = wpool.tile([P, N], i16, tag="a16")
                ot = opool.tile([P, N], i8, tag="ot")
                if do_mm:
                    ut = uts[(b * P) // CW]
                    boff = (b * P) % CW
                    loop = (
                        [(h, ks) for ks in range(0, KT, 2) for h in range(2)]
                        if ks_outer else
                        [(h, ks) for h in range(2) for ks in range(0, KT, 2)]
                    )
                    for h, ks in loop:
                        nc.tensor.matmul(
                            ps[:, h * 512:(h + 1) * 512],
                            ut[:, ks:ks + 2, boff:boff + P],
                            mt[:, ks:ks + 2, h * 512:(h + 1) * 512],
                            start=(ks == 0),
                            stop=(ks == KT - 2),
                            perf_mode=DR,
                            skip_group_check=ks_outer,
                        )
                if do_evict:
                    if evict == "outonly":
                        nc.sync.dma_start(y[b * P:(b + 1) * P, :], ot_shared[:])
                        continue
                    # W1: PSUM f32 -> i16
                    if ev_stage >= 1:
                        if evict == "w1dve" or (i % NB) >= w1_act:
                            nc.vector.tensor_copy(t16[:], ps[:])
                        else:
                            nc.scalar.activation(
                                t16[:], ps[:],
                                mybir.ActivationFunctionType.Copy,
                            )
                    # W2: AND with 1
                    if ev_stage >= 2:
                        nc.vector.tensor_scalar(
                            a16[:], t16[:], 1, None,
                            mybir.AluOpType.bitwise_and,
                        )
                    # W3 + output DMA
                    if ev_stage < 3:
                        continue
                    if evict == "i16out":
                        nc.sync.dma_start(y[b * P:(b + 1) * P, :], a16[:])
                    elif evict in ("dmacast", "w1dve"):
                        nc.gpsimd.dma_start(y[b * P:(b + 1) * P, :], a16[:])
                    else:
                        if evict == "dve" or (
                            evict == "split" and (i % NB) < w3_dve
                        ):
                            nc.vector.tensor_copy(ot[:], a16[:])
                        else:
                            nc.gpsimd.tensor_copy(ot[:], a16[:])
                        nc.sync.dma_start(y[b * P:(b + 1) * P, :], ot[:])
    nc.compile()
    return nc


def _build_nc_v2(reps=1, act_pairs=22, warm=(256, 256), main_chunk=1024,
                 chunk_bufs=3, wbufs=3, mt_splits=4):
    """v2: pair eviction ([128,2048] f32 = 4 PSUM banks per evict instr),
    i8 output, W1 split ACT/DVE, staged input DMA with pool backpressure.

    Per pair (2 b-tiles): 8 matmuls fill 4 banks; one W1 (PSUM f32->i16,
    ACT for act_pairs/32 of pairs else DVE), one DVE AND (i16), one DVE
    narrow (i16->i8, safe post-AND), 2 output DMAs.
    """
    import concourse.tile as tile
    from concourse import bacc, mybir

    nc = bacc.Bacc("TRN2", target_bir_lowering=False, debug=False)
    fp8 = mybir.dt.float8e4
    f32 = mybir.dt.float32
    i16 = mybir.dt.int16
    i8 = mybir.dt.int8
    DR = mybir.MatmulPerfMode.DoubleRow

    uT = nc.declare_dram_parameter("uT", [P, KT, SHARD], fp8, isOutput=False)
    mat = nc.declare_dram_parameter("mat", [P, KT, N], fp8, isOutput=False)
    # raw i16 sums; host computes & 1
    y = nc.declare_dram_parameter("y", [SHARD, N], i16, isOutput=True)

    # batch chunk schedule: warmup chunks then fixed-size main chunks
    chunks = list(warm)
    while sum(chunks) < SHARD:
        chunks.append(min(main_chunk, SHARD - sum(chunks)))
    starts = [sum(chunks[:i]) for i in range(len(chunks))]

    PAIRS = NB // 2

    with tile.TileContext(nc) as tc:
        with (
            tc.tile_pool(name="consts", bufs=1) as cpool,
            tc.tile_pool(name="uchunks", bufs=chunk_bufs) as upool,
            tc.tile_pool(name="work", bufs=wbufs) as wpool,
            tc.tile_pool(name="outs", bufs=wbufs) as opool,
            tc.tile_pool(name="psum", bufs=2, space="PSUM") as ppool,
        ):
            # mt as one DMA: [P, KT*N] rows are 4KB contiguous -> big packets
            mt = cpool.tile([P, KT, N], fp8, tag="mt")
            nc.sync.dma_start(mt[:], mat[:])
            # u chunk tiles from a small pool: chunk c+chunk_bufs's DMA
            # waits for chunk c's matmuls (natural backpressure keeps
            # early chunks from sharing DMA bandwidth with late ones)
            chunk_map = {}  # b-tile index -> (tile, local col offset)
            pending = list(zip(starts, chunks))

            def prefetch(upto_tile):
                # emit chunk DMAs for chunks whose first b-tile <= upto_tile;
                # warmup chunks come from consts pool (no reuse), main chunks
                # from upool (bufs=chunk_bufs gives DMA backpressure)
                for st, cw in pending[:]:
                    if st // P > upto_tile:
                        break
                    wi = starts.index(st)
                    pool = cpool if wi < len(warm) else upool
                    t = pool.tile([P, KT, cw], fp8,
                                  tag=("uw%d" % wi if wi < len(warm) else "uc"),
                                  name=f"uc{st}")
                    nc.sync.dma_start(t[:], uT[:, :, st:st + cw])
                    for bb in range(st // P, (st + cw) // P):
                        chunk_map[bb] = (t, bb * P - st)
                    pending.remove((st, cw))

            PF = 8  # prefetch distance in b-tiles

            for it in range(reps):
                for i in range(PAIRS):
                    prefetch(2 * i + 1 + PF)
                    ps = ppool.tile([P, 2 * N], f32, tag="ps", name="ps")
                    for t in range(2):
                        b = 2 * i + t
                        ut, boff = chunk_map[b]
                        for ks in range(0, KT, 2):
                            for h in range(2):
                                nc.tensor.matmul(
                                    ps[:, t * N + h * 512:
                                       t * N + (h + 1) * 512],
                                    ut[:, ks:ks + 2, boff:boff + P],
                                    mt[:, ks:ks + 2, h * 512:(h + 1) * 512],
                                    start=(ks == 0),
                                    stop=(ks == KT - 2),
                                    perf_mode=DR,
                                    skip_group_check=True,
                                )
                    t16 = wpool.tile([P, 2, N], i16, tag="t16")
                    # Bresenham split of W1 between ACT and DVE; raw sums
                    # go straight out (host does & 1)
                    on_act = (i * act_pairs) % PAIRS < act_pairs
                    if on_act:
                        nc.scalar.activation(
                            t16[:], ps[:],
                            mybir.ActivationFunctionType.Copy)
                    else:
                        nc.vector.tensor_copy(t16[:], ps[:])
                    for t in range(2):
                        b = 2 * i + t
                        nc.sync.dma_start(y[b * P:(b + 1) * P, :], t16[:, t])
    nc.compile()
    return nc


def chunk_schedule(warm, main_chunk):
    chunks = list(warm)
    while sum(chunks) < SHARD:
        chunks.append(min(main_chunk, SHARD - sum(chunks)))
    starts = [sum(chunks[:i]) for i in range(len(chunks))]
    return starts, chunks


def _build_nc_v3(reps=1, warm=(256, 256, 512), main_chunk=1024,
                 chunk_bufs=3, wbufs=3, warmup_mms=40, pf=8, out_u8=True,
                 pair_dma=False, chunk_major=False):
    """v3: pair PSUM ([128,2048] f32, bufs=2) with W1 split across BOTH
    engines per pair (ACT evicts tile A's 1024 cols, DVE tile B's) so the
    pair frees in ~1.4us < the 2.1us matmul fill time -> PE never stalls.
    Raw i16 sums out (host does &1). Dummy warmup matmuls during the input
    lead-in keep the PE's HAM clock at 2.4GHz for the first real tiles.
    """
    import concourse.tile as tile
    from concourse import bacc, mybir

    nc = bacc.Bacc("TRN2", target_bir_lowering=False, debug=False)
    fp8 = mybir.dt.float8e4
    f32 = mybir.dt.float32
    i16 = mybir.dt.int16
    DR = mybir.MatmulPerfMode.DoubleRow

    u8 = mybir.dt.uint8
    out_dt = u8 if out_u8 else i16

    # chunk_major: host lays u out chunk-contiguous ([P, KT*cw] per chunk,
    # concatenated) so each chunk DMA is one contiguous run per partition
    uT = nc.declare_dram_parameter(
        "uT", [P, KT * SHARD] if chunk_major else [P, KT, SHARD], fp8,
        isOutput=False)
    mat = nc.declare_dram_parameter("mat", [P, KT, N], fp8, isOutput=False)
    # raw sums out: u8 saturating (host certifies no 255 appeared -> exact,
    # else reruns the i16 build) or i16 exact
    y = nc.declare_dram_parameter("y", [SHARD, N], out_dt, isOutput=True)

    starts, chunks = chunk_schedule(warm, main_chunk)
    PAIRS = NB // 2

    with tile.TileContext(nc) as tc:
        with (
            tc.tile_pool(name="consts", bufs=1) as cpool,
            tc.tile_pool(name="uchunks", bufs=chunk_bufs) as upool,
            tc.tile_pool(name="work", bufs=wbufs) as wpool,
            tc.tile_pool(name="psum", bufs=4, space="PSUM") as ppool,
        ):
            # PE warmup: dummy matmuls with no DMA deps keep the HAM busy
            # window hot while inputs stream in. Scratch operands from a
            # memset tile; results land in a psum buf that a later tile
            # overwrites (start=True).
            if warmup_mms:
                scratch = cpool.tile([P, 2, 512], fp8, tag="scratch")
                nc.any.memset(scratch[:], 0)
                wp = ppool.tile([P, N], f32, tag="ps", name="ps_warm")
                for _ in range(warmup_mms):
                    nc.tensor.matmul(wp[:, 0:512], scratch[:, :, 0:P],
                                     scratch[:], start=True, stop=True,
                                     perf_mode=DR, skip_group_check=True)

            mt = cpool.tile([P, KT, N], fp8, tag="mt")
            nc.sync.dma_start(mt[:], mat[:])

            chunk_map = {}
            pending = list(zip(starts, chunks))

            def prefetch(upto_tile):
                for st, cw in pending[:]:
                    if st // P > upto_tile:
                        break
                    wi = starts.index(st)
                    pool = cpool if wi < len(warm) else upool
                    t = pool.tile([P, KT, cw], fp8,
                                  tag=("uw%d" % wi if wi < len(warm) else "uc"),
                                  name=f"uc{st}")
                    if chunk_major:
                        off = KT * st
                        src = uT[:, off:off + KT * cw].rearrange(
                            "p (k c) -> p k c", k=KT)
                    else:
                        src = uT[:, :, st:st + cw]
                    nc.sync.dma_start(t[:], src)
                    for bb in range(st // P, (st + cw) // P):
                        chunk_map[bb] = (t, bb * P - st)
                    pending.remove((st, cw))

            for it in range(reps):
                for b in range(NB):
                    prefetch(b + pf)
                    ps = ppool.tile([P, N], f32, tag="ps", name="ps")
                    ut, boff = chunk_map[b]
                    for ks in range(0, KT, 2):
                        for h in range(2):
                            nc.tensor.matmul(
                                ps[:, h * 512:(h + 1) * 512],
                                ut[:, ks:ks + 2, boff:boff + P],
                                mt[:, ks:ks + 2, h * 512:(h + 1) * 512],
                                start=(ks == 0),
                                stop=(ks == KT - 2),
                                perf_mode=DR,
                                skip_group_check=True,
                            )
                    # W1 alternates engines per tile; 4-deep psum pipeline
                    # absorbs eviction latency jitter
                    if pair_dma:
                        if b % 2 == 0:
                            t16p = wpool.tile([P, 2, N], out_dt, tag="t16")
                            nc.scalar.activation(
                                t16p[:, 0], ps[:],
                                mybir.ActivationFunctionType.Copy)
                        else:
                            nc.vector.tensor_copy(t16p[:, 1], ps[:])
                            dst = y[(b - 1) * P:(b + 1) * P, :].rearrange(
                                "(t p) n -> p t n", t=2)
                            nc.sync.dma_start(dst, t16p[:])
                        continue
                    t16 = wpool.tile([P, N], out_dt, tag="t16")
                    if b % 2 == 0:
                        nc.scalar.activation(t16[:], ps[:],
                                             mybir.ActivationFunctionType.Copy)
                    else:
                        nc.vector.tensor_copy(t16[:], ps[:])
                    nc.sync.dma_start(y[b * P:(b + 1) * P, :], t16[:])
    nc.compile()
    return nc


EVICT = "i16out"
W1_ACT = 48      # 48/64 PSUM->i16 converts on ACT, 16/64 on DVE
U_CHUNKS = 8     # input u loaded in 8 chunks so matmuls start early
WBUFS = 6
KS_OUTER = True  # k-pair outer loop: one LDWEIGHTS serves both psum halves

VERSION = 5
V2_OPTS = dict(act_pairs=17, warm=(256, 256, 512), main_chunk=1024,
               chunk_bufs=3, wbufs=3, mt_splits=1)
V3_OPTS = dict(warm=(256, 256, 512), main_chunk=1024,
               chunk_bufs=3, wbufs=6, warmup_mms=12, pf=8)
V3_OPTS_B = dict(warm=(128, 256, 512, 1024), main_chunk=1024,
                 chunk_bufs=3, wbufs=8, warmup_mms=22, pf=8)
V3_OPTS_C = dict(warm=(128, 256, 512), main_chunk=2048,
                 chunk_bufs=3, wbufs=8, warmup_mms=10, pf=12)
V3_OPTS_D = dict(warm=(128, 256, 512), main_chunk=2048,
                 chunk_bufs=3, wbufs=8, warmup_mms=6, pf=12)
V3_OPTS_E = dict(warm=(128, 256, 512), main_chunk=2048,
                 chunk_bufs=3, wbufs=4, warmup_mms=10, pf=12, pair_dma=True)
V3_OPTS_F = dict(warm=(128, 256, 512), main_chunk=2048,
                 chunk_bufs=3, wbufs=8, warmup_mms=10, pf=12,
                 chunk_major=True)


def _active_opts():
    return {5: V3_OPTS_C, 6: V3_OPTS_D, 7: V3_OPTS_E, 8: V3_OPTS_F,
            4: V3_OPTS_B, 3: V3_OPTS}.get(VERSION, V3_OPTS_C)


def get_nc(reps=1, out_u8=True):
    if VERSION == 8:
        key = (8, reps, tuple(sorted(V3_OPTS_F.items())), out_u8)
        if key not in _nc_cache:
            _nc_cache[key] = _build_nc_v3(reps, out_u8=out_u8, **V3_OPTS_F)
        return _nc_cache[key]
    if VERSION == 7:
        key = (7, reps, tuple(sorted(V3_OPTS_E.items())), out_u8)
        if key not in _nc_cache:
            _nc_cache[key] = _build_nc_v3(reps, out_u8=out_u8, **V3_OPTS_E)
        return _nc_cache[key]
    if VERSION == 6:
        key = (6, reps, tuple(sorted(V3_OPTS_D.items())), out_u8)
        if key not in _nc_cache:
            _nc_cache[key] = _build_nc_v3(reps, out_u8=out_u8, **V3_OPTS_D)
        return _nc_cache[key]
    if VERSION == 5:
        key = (5, reps, tuple(sorted(V3_OPTS_C.items())), out_u8)
        if key not in _nc_cache:
            _nc_cache[key] = _build_nc_v3(reps, out_u8=out_u8, **V3_OPTS_C)
        return _nc_cache[key]
    if VERSION == 4:
        key = (4, reps, tuple(sorted(V3_OPTS_B.items())), out_u8)
        if key not in _nc_cache:
            _nc_cache[key] = _build_nc_v3(reps, out_u8=out_u8, **V3_OPTS_B)
        return _nc_cache[key]
    if VERSION == 3:
        key = (3, reps, tuple(sorted(V3_OPTS.items())), out_u8)
        if key not in _nc_cache:
            _nc_cache[key] = _build_nc_v3(reps, out_u8=out_u8, **V3_OPTS)
        return _nc_cache[key]
    if VERSION == 2:
        key = (2, reps, tuple(sorted(V2_OPTS.items())))
        if key not in _nc_cache:
            _nc_cache[key] = _build_nc_v2(reps, **V2_OPTS)
        return _nc_cache[key]
    key = (reps, EVICT, W1_ACT, U_CHUNKS, WBUFS, KS_OUTER)
    if key not in _nc_cache:
        _nc_cache[key] = _build_nc(reps, evict=EVICT, w1_act=W1_ACT,
                                   u_chunks=U_CHUNKS, wbufs=WBUFS,
                                   ks_outer=KS_OUTER)
    return _nc_cache[key]


def _to_k_major(a_km, free):
    """[K, free] -> [P, KT, free] with k = ks*128 + p."""
    return np.ascontiguousarray(
        a_km.reshape(KT, P, free).transpose(1, 0, 2)
    )


def make_in_maps(u, M):
    u8 = np.asarray(u).astype(FP8_NP)
    m8 = np.asarray(M).astype(FP8_NP)
    mat3 = _to_k_major(m8, N)
    opts = _active_opts() if VERSION >= 3 else {}
    chunk_major = bool(opts.get("chunk_major"))
    if chunk_major:
        starts, chunks = chunk_schedule(opts["warm"], opts["main_chunk"])
    in_maps = []
    for i in range(N_CORES):
        uT_i = np.ascontiguousarray(u8[i * SHARD:(i + 1) * SHARD, :].T)
        uk = _to_k_major(uT_i, SHARD)  # [P, KT, SHARD]
        if chunk_major:
            uk = np.concatenate(
                [uk[:, :, st:st + cw].reshape(P, KT * cw)
                 for st, cw in zip(starts, chunks)], axis=1)
        in_maps.append({"uT": uk, "mat": mat3})
    return in_maps


def kernel(u, crc_gen, info_pos, ind_gather, perm_out):
    from concourse.bass_utils import run_bass_kernel_spmd

    M = build_M(crc_gen, info_pos, ind_gather, perm_out)
    in_maps = make_in_maps(u, M)
    nc = get_nc()
    res = run_bass_kernel_spmd(nc, in_maps, core_ids=list(range(N_CORES)))
    ys = [np.asarray(r["y"]) for r in res.results]
    if ys[0].dtype == np.uint8 and any((yc == 255).any() for yc in ys):
        # saturation certificate failed (a sum may have clipped at 255):
        # rerun with exact i16 output
        nc16 = get_nc(out_u8=False)
        res = run_bass_kernel_spmd(nc16, in_maps,
                                   core_ids=list(range(N_CORES)))
        ys = [np.asarray(r["y"]) for r in res.results]
    out = np.concatenate([(yc & 1).astype(np.float32) for yc in ys], axis=0)
    return out

